# revision 2
# baseline (speedup 1.0000x reference)
"""Binarized conv block (BinBlock) Trainium2 Bass kernel — fp16, 9-wave.

Reference computation (per image):
    xb    = sign(x)                                  # +/-1
    alpha = mean|W| over (I,kh,kw)                   # [O]
    wb    = alpha * sign(W)
    xp    = pad(xb, 1, value=-1)
    out   = conv2d(xp, wb) + bias
    out   = out*gBN + (beta - mean*gBN),  gBN = gamma/sqrt(var+eps)
    out   = out + x

Kernel algebra: let s = alpha*gBN, b2 = bias*gBN + beta - mean*gBN.
    out = conv2d(pad(sign(x),-1), sign(W)*2s) * 0.5 + b2 + x
Activations binarize to {+0.5,-0.5} (exact in fp16; pad = -0.5), weight
entries are +/-2s[o] (fp16), so every PE product is +/-fp16(2s)/2 exactly and
the PSUM column sums k*(fp16(2s)/2) are exact in fp32: psum = s'*conv_int.

The residual is NOT a matmul here (the v1 kernel spent 1 of its 10 PE waves
injecting x via a diag(1/(2s)) matmul).  Instead x is pre-copied into the
fp16 staging tiles by on-chip SBUF->SBUF DMAs (DMA is address-based, so the
(img,ch) -> (half,ch) partition relayout is free), and the epilogue becomes
    tmp = psum + b2                (ScalarE activation, bias=b2)
    st  = tmp + st(=x)             (DVE tensor_tensor, in-place)
The 9 conv taps then run as plain fp16 matmuls streaming N=448 over the four
64x64 PE quadrants (9 waves/slot instead of 10 -> ~10% less PE time).
Engine budget per slot (~1.7us PE pace): ScalarE 2x752ns act, DVE 2x397ns
tt-add + ~760ns sign: both fit under the PE pace.

fp8 DoubleRow was re-examined and is definitively closed on this toolchain:
only P=128/M=128 ktile-major DR compiles (P=64 and M=64 variants fail walrus
ISA codegen; interleaved weights and byte-stride ktiles crash), and with 64
output channels an M=128 DR instruction can only be block-diagonal, which
wastes exactly the 2x it would win.

I/O is fp16 end-to-end (host converts, rel err ~3e-4): DRAM layouts are
[pair][128][H*W] with partition = (img-in-pair)*64 + channel, so every DMA
descriptor is a multi-KB contiguous span.  Inputs stream on the sync HWDGE
ring in row chunks so conv starts during the load; outputs stage per image
in SBUF and go out in progressively finer chunks.

Schedule notes:
  - All consts are packed into ONE byte tile + bitcast views; the const DMA
    rides the scalar HWDGE ring so it does not delay the first x chunk on
    the sync ring.
  - Dep-free dummy matmuls on a memset tile warm the PE HAM clock gate
    (1.2 -> 2.4 GHz) during the ~11us startup window.
  - x pre-copies are split per (img, hf, m-range) and emitted just-in-time
    across the pair's slots so their chunk-gating semaphore waits never
    head-of-line block the issuing queue (gpsimd for even m-ranges, scalar
    for odd) and never race the epilogue that reads them.
  - Next pair's sign ops are spread over slots m=4..6 so the strict-FIFO
    DVE queue never stalls an epilogue add behind an input-DMA wait.

Measured on trn2 (8 cores, axon): v1 baseline 73.4us; this version ~66us.
Note the chip occasionally sits in a uniformly 1.2x slower clock state for a
run or two; compare runs via MATMUL median duration (~349ns fast, ~418 slow).
"""

import numpy as np

import concourse.bass as bass
import concourse.bacc as bacc
import concourse.tile as tile
import concourse.mybir as mybir
from concourse import bass_utils

F32 = mybir.dt.float32
F16 = mybir.dt.float16
U8 = mybir.dt.uint8

B, C, H, W = 32, 64, 112, 112
NCORES = 8
BSH = B // NCORES          # images per core
HWF = H * W                # 12544
HP = H + 2                 # 114 padded
SGW = HP * HP              # 12996
NB = 4 * W                 # 448 (one PSUM bank: 512 fp32)
NSLOT = 14                 # (m,j) slots per image
BN_EPS = 1e-5

ACT_ID = mybir.ActivationFunctionType.Identity
OP_GE = mybir.AluOpType.is_ge
OP_SUB = mybir.AluOpType.subtract
OP_MULT = mybir.AluOpType.mult
OP_ADD = mybir.AluOpType.add


def build_kernel_body(tc, out_d, x_d, cs_d):
    nc = tc.nc
    with (
        tc.tile_pool(name="const", bufs=1) as constp,
        tc.tile_pool(name="warmup", bufs=1) as warmupp,
        tc.tile_pool(name="xraw", bufs=2) as xrawp,
        tc.tile_pool(name="sign", bufs=2) as signp,
        tc.tile_pool(name="stage", bufs=4) as stagep,
        tc.tile_pool(name="tmp", bufs=4) as tmpp,
        tc.tile_pool(name="psum", bufs=8, space="PSUM") as psump,
    ):
        # consts in one byte tile; DMA on the scalar ring so the sync ring's
        # first x chunk is not queued behind it
        ct = constp.tile([128, 1156], U8)
        nc.scalar.dma_start(ct[:], cs_d[:])
        ws_t = ct[:, 0:1152].bitcast(F16)     # (+/-2s)*sign(W)^T  [128, 576]
        b2_t = ct[:, 1152:1156].bitcast(F32)  # b2                 [128, 1]

        # PE warm-up: dep-free dummy matmuls keep the HAM activity monitor
        # busy during startup so the first real matmuls run at full clock
        wm = warmupp.tile([64, 520], F16)
        nc.gpsimd.memset(wm[:], 0.5)
        wps = psump.tile([128, NB], F32, name="ps_warm", tag="ps")
        for _ in range(6):
            nc.tensor.matmul(
                wps[0:8, :], wm[:, 512:520], wm[:, 0:448],
                start=True, stop=True, skip_group_check=True,
            )

        CHUNKS = ((0, 12), (12, 20), (20, 48), (48, 80), (80, H))
        OUT_CUTS = {(2, 1): (0, 6), (4, 1): (6, 10), (6, 0): (10, 13), (6, 1): (13, 14)}
        # x pre-copy m-ranges and the (m, j) slot of the OWNING pair at which
        # each is emitted ("L" = at load time, i.e. right after the DMAs)
        PRECOPY = {"L": (0, 1), (0, 0): (1, 3), (1, 0): (3, 5), (2, 0): (5, 7)}

        def chunk_dma(p, xr, ci, eng):
            ra, rb = CHUNKS[ci]
            eng.dma_start(xr[:, ra * W : rb * W], x_d[p, :, ra * W : rb * W])

        def chunk_sign(p, xr, sg3, ci):
            # binarize one row chunk: (x >= 0) - 0.5  ->  {+0.5, -0.5}
            ra, rb = CHUNKS[ci]
            xr3 = xr[:].rearrange("p (h w) -> p h w", w=W)
            nc.vector.tensor_scalar(
                sg3[:, 1 + ra : 1 + rb, 1 : HP - 1],
                xr3[:, ra:rb, :],
                0.0,
                0.5,
                OP_GE,
                OP_SUB,
            )

        def load_pair_dmas(p):
            xr = xrawp.tile([128, HWF], F16, name=f"xr_{p}", tag="xr")
            sg = signp.tile([128, SGW], F16, name=f"sg_{p}", tag="sg")
            sts = [
                stagep.tile([128, NSLOT * NB], F16, name=f"st_p{p}i{ih}", tag="st")
                for ih in range(2)
            ]
            sg3 = sg[:].rearrange("p (h w) -> p h w", w=HP)
            for ci in range(len(CHUNKS)):
                chunk_dma(p, xr, ci, nc.sync)
            # -0.5 padding border (top/bottom rows, left/right columns)
            nc.gpsimd.memset(sg3[:, 0, :], -0.5)
            nc.gpsimd.memset(sg3[:, HP - 1, :], -0.5)
            nc.gpsimd.memset(sg3[:, 1 : HP - 1, 0], -0.5)
            nc.gpsimd.memset(sg3[:, 1 : HP - 1, HP - 1], -0.5)
            return xr, sg, sg3, sts

        def precopy(xr, sts, ma, mb):
            # x -> staging tiles, relayout (img,ch)->(hf,ch) via on-chip DMA.
            # blk = 4m + 2hf + j lives at xr[ih*64+c, blk*448:...]; slot
            # (2m+j) of image ih at st[hf*64+c, (2m+j)*448:...].
            xr5 = xr[:].rearrange("p (m h j w) -> p m h j w", h=2, j=2, w=NB)
            for ih in range(2):
                st6 = sts[ih][:].rearrange("p (m j w) -> p m j w", j=2, w=NB)
                for hf in range(2):
                    eng = nc.gpsimd if (hf == 0) else nc.scalar
                    eng.dma_start(
                        st6[hf * 64 : hf * 64 + 64, ma:mb, :, :],
                        xr5[ih * 64 : ih * 64 + 64, ma:mb, hf, :, :],
                    )

        # prologue: pair 0 loads + signs + early precopies up-front
        pro = {}
        xr, sg, sg3, sts0 = load_pair_dmas(0)
        for ci in range(len(CHUNKS)):
            chunk_sign(0, xr, sg3, ci)
        precopy(xr, sts0, *PRECOPY["L"])
        precopy(xr, sts0, 1, 3)
        precopy(xr, sts0, 3, 5)
        precopy(xr, sts0, 5, 7)
        pro[0] = (xr, sg, sg3, sts0)

        for p in range(BSH // 2):  # image pairs; image 2p -> partitions 0:64
            xr, sg, sg3, sts = pro.pop(p)
            nxt = None
            for m in range(7):
                for j in range(2):
                    psb = [
                        psump.tile(
                            [128, NB], F32, name=f"ps_p{p}m{m}j{j}i{ih}", tag="ps"
                        )
                        for ih in range(2)
                    ]
                    # 9 conv taps, round-robin over the 4 array quadrants
                    for pos in range(9):
                        dh, dw = divmod(pos, 3)
                        for q in range(4):
                            ih, hf = divmod(q, 2)
                            blk = 4 * m + 2 * hf + j
                            r0 = 4 * blk + dh
                            nc.tensor.matmul(
                                psb[ih][64 * hf : 64 * hf + 64, :],
                                ws_t[64 * ih : 64 * ih + 64, 64 * pos : 64 * pos + 64],
                                sg3[64 * ih : 64 * ih + 64, r0 : r0 + 4, dw : dw + W],
                                start=(pos == 0),
                                stop=(pos == 8),
                                skip_group_check=True,
                            )
                    # epilogue: tmp = psum + b2 (ScalarE), st = tmp + x (DVE)
                    for ih in range(2):
                        dst = sts[ih][:, (2 * m + j) * NB : (2 * m + j + 1) * NB]
                        tmp = tmpp.tile([128, NB], F16, name=f"tp_p{p}m{m}j{j}i{ih}", tag="tp")
                        nc.scalar.activation(tmp[:], psb[ih][:, :], ACT_ID, bias=b2_t[:, 0:1])
                        nc.vector.tensor_tensor(dst, tmp[:], dst, OP_ADD)
                    # stream each image out in progressively finer DMA chunks
                    cut = OUT_CUTS.get((m, j))
                    if cut:
                        lo, hi = (c * NB for c in cut)
                        last = (m, j) == (6, 1)
                        for ih in range(2):
                            n = 2 * p + ih
                            eng = nc.scalar if (last and ih == 0) else nc.gpsimd
                            eng.dma_start(out_d[n, :, lo:hi], sts[ih][:, lo:hi])
                    # next pair: DMAs at m=3; signs and precopies spread over
                    # m=4..6 so chunk-gated waits never stall a queue
                    if p + 1 < BSH // 2:
                        if (m, j) == (3, 0):
                            nxt = load_pair_dmas(p + 1)
                            precopy(nxt[0], nxt[3], *PRECOPY["L"])
                        elif (m, j) == (4, 0):
                            for ci in (0, 1):
                                chunk_sign(p + 1, nxt[0], nxt[2], ci)
                        elif (m, j) == (4, 1):
                            chunk_sign(p + 1, nxt[0], nxt[2], 2)
                            precopy(nxt[0], nxt[3], 1, 3)
                        elif (m, j) == (5, 0):
                            chunk_sign(p + 1, nxt[0], nxt[2], 3)
                        elif (m, j) == (5, 1):
                            chunk_sign(p + 1, nxt[0], nxt[2], 4)
                            precopy(nxt[0], nxt[3], 3, 5)
                        elif (m, j) == (6, 0):
                            precopy(nxt[0], nxt[3], 5, 7)
            if nxt is not None:
                pro[p + 1] = nxt


def build_nc():
    nc = bacc.Bacc(trn_type="TRN2", debug=False, num_devices=NCORES)
    x_d = nc.dram_tensor("x", [BSH // 2, 128, HWF], F16, kind="ExternalInput")
    cs_d = nc.dram_tensor("consts", [128, 1156], U8, kind="ExternalInput")
    out_d = nc.dram_tensor("out", [BSH, 128, NSLOT * NB], F16, kind="ExternalOutput")
    with tile.TileContext(nc) as tc:
        build_kernel_body(tc, out_d, x_d, cs_d)
    nc.compile()
    return nc


def prep_consts(weight, bias, gamma, beta, run_mean, run_var):
    """Host-side constant prep (numpy, fp64 for the folding math)."""
    w = np.asarray(weight, np.float64)
    alpha = np.mean(np.abs(w), axis=(1, 2, 3))            # [O]
    g = np.asarray(gamma, np.float64) / np.sqrt(np.asarray(run_var, np.float64) + BN_EPS)
    s = alpha * g                                          # [O]
    b2 = np.asarray(bias, np.float64) * g + np.asarray(beta, np.float64) - np.asarray(
        run_mean, np.float64
    ) * g

    # lhsT layout [I(dup to 128), tap, O]; entries sign(W)*2s[O] so PE
    # products with +/-0.5 sign data are exact fp16(2s)/2 halves
    wsc = np.sign(w) * (2.0 * s)[:, None, None, None]      # [O, I, 3, 3]
    ws = wsc.transpose(1, 2, 3, 0).reshape(C, 9 * C)
    ws128 = np.concatenate([ws, ws], axis=0).astype(np.float16)

    bi = np.concatenate([b2, b2]).astype(np.float32)[:, None]  # [128, 1]
    packed = np.concatenate(
        [
            np.ascontiguousarray(ws128).view(np.uint8),
            np.ascontiguousarray(bi).view(np.uint8),
        ],
        axis=1,
    )  # [128, 1156]
    return packed


_CACHE = {}


def kernel(x, weight, bias, gamma, beta, run_mean, run_var, _trace=False, _trace_kwargs=None):
    x = np.asarray(x)
    consts = prep_consts(weight, bias, gamma, beta, run_mean, run_var)
    # [core][pair][ih*64+c][h*w] in fp16
    x16 = np.ascontiguousarray(
        x.reshape(NCORES, BSH // 2, 128, HWF).astype(np.float16)
    )

    if "nc" not in _CACHE:
        _CACHE["nc"] = build_nc()
    nc = _CACHE["nc"]

    in_maps = [dict(x=x16[i], consts=consts) for i in range(NCORES)]
    res = bass_utils.run_bass_kernel_spmd(
        nc,
        in_maps,
        core_ids=list(range(NCORES)),
        trace=_trace,
        **(_trace_kwargs or {}),
    )
    outs = []
    for i in range(NCORES):
        o = np.asarray(res.results[i]["out"])  # [4, 128, 6272] fp16
        # partition=(hf,c), free=(m,j,r,w); row = m*16 + hf*8 + j*4 + r
        o = (
            o.reshape(BSH, 2, C, 7, 2, 4, W)
            .transpose(0, 2, 3, 1, 4, 5, 6)
            .reshape(BSH, C, H, W)
        )
        outs.append(o)
    out = np.concatenate(outs, axis=0).astype(np.float32)
    if _trace:
        kernel.last_results = res
    return out


# revision 3
# speedup vs baseline: 1.0655x; 1.0655x over previous
"""Binarized conv block (BinBlock) Trainium2 Bass kernel — fp16, 9-wave.

Reference computation (per image):
    xb    = sign(x);  alpha = mean|W| over (I,kh,kw)
    out   = conv2d(pad(xb,-1), alpha*sign(W)) + bias
    out   = out*gBN + (beta - mean*gBN) + x,   gBN = gamma/sqrt(var+eps)

Kernel algebra: let s = alpha*gBN, S2 = fp16(2s), b2 = bias*gBN + beta
- mean*gBN.  Activations binarize to b = (x>=0) in {0,1} (single-ALU-op
DVE sign; pad = 0), weights are S2[o]*sign(W) (fp16), so PE products are
{0, +/-S2} and PSUM column sums k*S2 are exact in fp32:
    psum = S2*sum(sign(W)*b) = s'*conv_int + 0.5*S2*sum(sign W)
    out  = psum + b2' + x,   b2' = b2 - 0.5*S2*sum_ct(sign W[o])  (host)

The residual x is NOT a matmul here (the v1 kernel spent 1 of its 10 PE
waves injecting x via a diag matmul; 9 waves/slot = ~10% less PE time).
Instead the epilogue is
    tmp = psum + b2'               (ScalarE activation, bias=b2')
    st  = tmp + x                  (DVE tensor_tensor, batched over m)
where the partition-aligned half of x ((img,ch) == (half,ch)) is read
straight from the raw-x tile and the crossed half is pre-copied into the
staging tiles by on-chip SBUF->SBUF DMAs (DMA is address-based, so the
partition relayout is free).  Only the crossed half is copied: a full
copy (6.4MB/core r+w) exceeds the ~368GB/s DMA-fabric budget that also
carries input and output (measured: full-copy variant runs 98us).
DVE adds are batched over m-ranges ((0,3),(3,5),(5,6)) because each DVE
op costs ~164ns fixed + ~0.52ns/elem: per-slot half-adds would put DVE
over the PE pace, batched ones leave ~0.2us/slot slack.  The last m=6
slots instead pre-copy BOTH halves (tiny) and run one fused DVE
scalar_tensor_tensor (st = (psum + b2') + st) per image so the tail
after the final matmul stays ~2us.

fp8 DoubleRow was re-examined and is definitively closed on this
toolchain: only P=128/M=128 ktile-major DR compiles (P=64 and M=64 fail
walrus ISA codegen; interleaved weights and byte-stride ktiles crash),
and with 64 output channels an M=128 DR instruction can only be
block-diagonal, which wastes exactly the 2x it would win.

I/O is fp16 end-to-end (host converts, rel err ~4e-4): DRAM layouts are
[pair][128][H*W] with partition = (img-in-pair)*64 + channel.  Inputs
stream on the sync HWDGE ring in row chunks; consts ride the scalar ring
so they don't delay the first chunk; outputs stage per image in SBUF and
drain in progressively finer cuts on the gpsimd/scalar rings.

Schedule notes:
  - Dep-free dummy matmuls on a memset tile warm the PE HAM clock gate
    (1.2 -> 2.4 GHz) during the ~11us startup window.
  - Next pair's input DMAs go out at (3,0); its sign ops and pre-copies
    are spread over slots (4,0)..(6,0) so their chunk-gated semaphore
    waits never head-of-line block a queue that carries epilogue work.

Measured on trn2 (8 cores, axon): v1 baseline 73.4us.  Note the chip
occasionally sits in a uniformly 1.2x slower clock state for a run or
two; compare runs via MATMUL median duration (~349ns fast, ~418 slow).
"""

import numpy as np

import concourse.bass as bass
import concourse.bacc as bacc
import concourse.tile as tile
import concourse.mybir as mybir
from concourse import bass_utils

F32 = mybir.dt.float32
F16 = mybir.dt.float16
U8 = mybir.dt.uint8

B, C, H, W = 32, 64, 112, 112
NCORES = 8
BSH = B // NCORES          # images per core
HWF = H * W                # 12544
HP = H + 2                 # 114 padded
SGW = HP * HP              # 12996
NB = 4 * W                 # 448 (one PSUM bank: 512 fp32)
NSLOT = 14                 # (m,j) slots per image
BN_EPS = 1e-5

ACT_ID = mybir.ActivationFunctionType.Identity
OP_GE = mybir.AluOpType.is_ge
OP_SUB = mybir.AluOpType.subtract
OP_MULT = mybir.AluOpType.mult
OP_ADD = mybir.AluOpType.add


def build_kernel_body(tc, out_d, x_d, cs_d):
    nc = tc.nc
    with (
        tc.tile_pool(name="const", bufs=1) as constp,
        tc.tile_pool(name="warmup", bufs=1) as warmupp,
        tc.tile_pool(name="xraw", bufs=2) as xrawp,
        tc.tile_pool(name="sign", bufs=2) as signp,
        tc.tile_pool(name="stage", bufs=4) as stagep,
        tc.tile_pool(name="tmp", bufs=6) as tmpp,
        tc.tile_pool(name="psum", bufs=8, space="PSUM") as psump,
    ):
        # consts in one byte tile; DMA on the scalar ring so the sync ring's
        # first x chunk is not queued behind it
        ct = constp.tile([128, 1156], U8)
        nc.scalar.dma_start(ct[:], cs_d[:])
        ws_t = ct[:, 0:1152].bitcast(F16)     # S2*sign(W)^T  [128, 576]
        b2_t = ct[:, 1152:1156].bitcast(F32)  # b2'           [128, 1]

        # PE warm-up: dep-free dummy matmuls keep the HAM activity monitor
        # busy during startup so the first real matmuls run at full clock
        wm = warmupp.tile([64, 520], F16)
        nc.gpsimd.memset(wm[:], 0.5)
        wps = psump.tile([128, NB], F32, name="ps_warm", tag="ps")
        for _ in range(6):
            nc.tensor.matmul(
                wps[0:8, :], wm[:, 512:520], wm[:, 0:448],
                start=True, stop=True, skip_group_check=True,
            )

        CHUNKS = ((0, 12), (12, 20), (20, 48), (48, 80), (80, H))
        # output cuts: slot ranges emitted at the given (m, j)
        OUT_CUTS = {(2, 1): (0, 6), (4, 1): (6, 10), (6, 0): (10, 12),
                    (6, 1): (12, 13), "end": (13, 14)}
        # DVE-add batches (m-ranges) for m<6; m=6 uses fused stt per slot
        BATCHES = ((0, 3), (3, 5), (5, 6))

        def chunk_dma(p, xr, ci):
            ra, rb = CHUNKS[ci]
            nc.sync.dma_start(xr[:, ra * W : rb * W], x_d[p, :, ra * W : rb * W])

        def chunk_sign(xr, sg3, ci):
            # binarize one row chunk: b = (x >= 0) in {0, 1}
            ra, rb = CHUNKS[ci]
            xr3 = xr[:].rearrange("p (h w) -> p h w", w=W)
            nc.vector.tensor_scalar(
                sg3[:, 1 + ra : 1 + rb, 1 : HP - 1],
                xr3[:, ra:rb, :],
                0.0,
                None,
                OP_GE,
            )

        def load_pair_dmas(p):
            xr = xrawp.tile([128, HWF], F16, name=f"xr_{p}", tag="xr")
            sg = signp.tile([128, SGW], F16, name=f"sg_{p}", tag="sg")
            sts = [
                stagep.tile([128, NSLOT * NB], F16, name=f"st_p{p}i{ih}", tag="st")
                for ih in range(2)
            ]
            sg3 = sg[:].rearrange("p (h w) -> p h w", w=HP)
            for ci in range(len(CHUNKS)):
                chunk_dma(p, xr, ci)
            # pad border b=0 (== sign -1 under the {0,1} encoding)
            nc.gpsimd.memset(sg3[:, 0, :], 0.0)
            nc.gpsimd.memset(sg3[:, HP - 1, :], 0.0)
            nc.gpsimd.memset(sg3[:, 1 : HP - 1, 0], 0.0)
            nc.gpsimd.memset(sg3[:, 1 : HP - 1, HP - 1], 0.0)
            return xr, sg, sg3, sts

        def precopy_crossed(xr, sts, ma, mb, m6_full=False):
            # crossed half of x -> staging tiles via on-chip DMA: image ih's
            # x lives at partitions ih*64+c but its hf=(1-ih) output half
            # stages at partitions (1-ih)*64+c.  blk = 4m + 2hf + j.
            xr5 = xr[:].rearrange("p (m h j w) -> p m h j w", h=2, j=2, w=NB)
            for ih in range(2):
                st6 = sts[ih][:].rearrange("p (m j w) -> p m j w", j=2, w=NB)
                hfs = (0, 1) if m6_full else (1 - ih,)
                for hf in hfs:
                    eng = nc.gpsimd if ((ih + hf) % 2 == 0) else nc.scalar
                    eng.dma_start(
                        st6[hf * 64 : hf * 64 + 64, ma:mb, :, :],
                        xr5[ih * 64 : ih * 64 + 64, ma:mb, hf, :, :],
                    )

        def batch_add(xr, sts, tmps, ma, mb):
            # st[...] += tmp (aligned half also += x straight from xr)
            n = mb - ma
            xr5 = xr[:].rearrange("p (m h j w) -> p m h j w", h=2, j=2, w=NB)
            for ih in range(2):
                st6 = sts[ih][:].rearrange("p (m j w) -> p m j w", j=2, w=NB)
                tm6 = tmps[ih][:].rearrange("p (m j w) -> p m j w", j=2, w=NB)
                al, cr = ih, 1 - ih  # aligned hf == ih
                # aligned: st = tmp + x (x read in place from xr)
                nc.vector.tensor_tensor(
                    st6[al * 64 : al * 64 + 64, ma:mb, :, :],
                    tm6[al * 64 : al * 64 + 64, 0:n, :, :],
                    xr5[ih * 64 : ih * 64 + 64, ma:mb, al, :, :],
                    OP_ADD,
                )
                # crossed: st(=precopied x) += tmp, in place
                nc.vector.tensor_tensor(
                    st6[cr * 64 : cr * 64 + 64, ma:mb, :, :],
                    tm6[cr * 64 : cr * 64 + 64, 0:n, :, :],
                    st6[cr * 64 : cr * 64 + 64, ma:mb, :, :],
                    OP_ADD,
                )

        # prologue: pair 0 loads + signs + pre-copies up-front
        pro = {}
        xr0, sg0, sg30, sts0 = load_pair_dmas(0)
        for ci in range(len(CHUNKS)):
            chunk_sign(xr0, sg30, ci)
        precopy_crossed(xr0, sts0, 0, 3)
        precopy_crossed(xr0, sts0, 3, 5)
        precopy_crossed(xr0, sts0, 5, 6)
        precopy_crossed(xr0, sts0, 6, 7, m6_full=True)
        pro[0] = (xr0, sg0, sg30, sts0)

        for p in range(BSH // 2):  # image pairs; image 2p -> partitions 0:64
            xr, sg, sg3, sts = pro.pop(p)
            nxt = None
            tmps = None
            for m in range(7):
                bat = next(((a, b) for (a, b) in BATCHES if a <= m < b), None)
                for j in range(2):
                    if m < 6 and (m, j) == (bat[0], 0):
                        blen = bat[1] - bat[0]
                        tmps = [
                            tmpp.tile([128, blen * 2 * NB], F16,
                                      name=f"tp_p{p}b{bat[0]}i{ih}", tag="tp")
                            for ih in range(2)
                        ]
                    psb = [
                        psump.tile(
                            [128, NB], F32, name=f"ps_p{p}m{m}j{j}i{ih}", tag="ps"
                        )
                        for ih in range(2)
                    ]
                    # 9 conv taps, round-robin over the 4 array quadrants
                    for pos in range(9):
                        dh, dw = divmod(pos, 3)
                        for q in range(4):
                            ih, hf = divmod(q, 2)
                            blk = 4 * m + 2 * hf + j
                            r0 = 4 * blk + dh
                            nc.tensor.matmul(
                                psb[ih][64 * hf : 64 * hf + 64, :],
                                ws_t[64 * ih : 64 * ih + 64, 64 * pos : 64 * pos + 64],
                                sg3[64 * ih : 64 * ih + 64, r0 : r0 + 4, dw : dw + W],
                                start=(pos == 0),
                                stop=(pos == 8),
                                skip_group_check=True,
                            )
                    if m < 6:
                        # epilogue part 1: tmp = psum + b2' (ScalarE)
                        for ih in range(2):
                            lo = (2 * (m - bat[0]) + j) * NB
                            nc.scalar.activation(
                                tmps[ih][:, lo : lo + NB], psb[ih][:, :],
                                ACT_ID, bias=b2_t[:, 0:1],
                            )
                        # epilogue part 2 at batch end: batched DVE adds
                        if (m, j) == (bat[1] - 1, 1):
                            batch_add(xr, sts, tmps, bat[0], bat[1])
                    else:
                        # m=6: both halves of x are pre-copied into st; one
                        # fused DVE op per image: st = (psum + b2') + st
                        for ih in range(2):
                            dst = sts[ih][:, (2 * m + j) * NB : (2 * m + j + 1) * NB]
                            nc.vector.scalar_tensor_tensor(
                                dst, psb[ih][:, :], b2_t[:, 0:1], dst,
                                OP_ADD, OP_ADD,
                            )
                    # stream each image out in progressively finer DMA cuts
                    cut = OUT_CUTS.get((m, j))
                    if cut:
                        lo, hi = (c * NB for c in cut)
                        last = (m, j) == (6, 1)
                        for ih in range(2):
                            n = 2 * p + ih
                            eng = nc.scalar if (last and ih == 0) else nc.gpsimd
                            eng.dma_start(out_d[n, :, lo:hi], sts[ih][:, lo:hi])
                    # next pair: DMAs at (3,0); signs and pre-copies spread
                    # over (4,0)..(6,0) so chunk-gated waits never stall a
                    # queue carrying epilogue work
                    if p + 1 < BSH // 2:
                        if (m, j) == (3, 0):
                            nxt = load_pair_dmas(p + 1)
                        elif (m, j) == (4, 0):
                            for ci in (0, 1):
                                chunk_sign(nxt[0], nxt[2], ci)
                        elif (m, j) == (4, 1):
                            chunk_sign(nxt[0], nxt[2], 2)
                            precopy_crossed(nxt[0], nxt[3], 0, 3)
                        elif (m, j) == (5, 0):
                            chunk_sign(nxt[0], nxt[2], 3)
                        elif (m, j) == (5, 1):
                            chunk_sign(nxt[0], nxt[2], 4)
                            precopy_crossed(nxt[0], nxt[3], 3, 5)
                        elif (m, j) == (6, 0):
                            precopy_crossed(nxt[0], nxt[3], 5, 6)
                            precopy_crossed(nxt[0], nxt[3], 6, 7, m6_full=True)
            # final small cut for this pair (slot 13) after the last stt
            lo, hi = (c * NB for c in OUT_CUTS["end"])
            for ih in range(2):
                n = 2 * p + ih
                eng = nc.gpsimd if ih == 0 else nc.scalar
                eng.dma_start(out_d[n, :, lo:hi], sts[ih][:, lo:hi])
            if nxt is not None:
                pro[p + 1] = nxt


def build_nc():
    nc = bacc.Bacc(trn_type="TRN2", debug=False, num_devices=NCORES)
    x_d = nc.dram_tensor("x", [BSH // 2, 128, HWF], F16, kind="ExternalInput")
    cs_d = nc.dram_tensor("consts", [128, 1156], U8, kind="ExternalInput")
    out_d = nc.dram_tensor("out", [BSH, 128, NSLOT * NB], F16, kind="ExternalOutput")
    with tile.TileContext(nc) as tc:
        build_kernel_body(tc, out_d, x_d, cs_d)
    nc.compile()
    return nc


def prep_consts(weight, bias, gamma, beta, run_mean, run_var):
    """Host-side constant prep (numpy, fp64 for the folding math)."""
    w = np.asarray(weight, np.float64)
    alpha = np.mean(np.abs(w), axis=(1, 2, 3))            # [O]
    g = np.asarray(gamma, np.float64) / np.sqrt(np.asarray(run_var, np.float64) + BN_EPS)
    s = alpha * g                                          # [O]
    b2 = np.asarray(bias, np.float64) * g + np.asarray(beta, np.float64) - np.asarray(
        run_mean, np.float64
    ) * g

    # lhsT layout [I(dup to 128), tap, O]; entries S2*sign(W), S2 = fp16(2s):
    # products with b in {0,1} are exactly {0, +/-S2}
    S2 = np.float16(2.0 * s).astype(np.float64)            # [O]
    wsign = np.sign(w)                                     # [O, I, 3, 3]
    wsc = wsign * S2[:, None, None, None]
    ws = wsc.transpose(1, 2, 3, 0).reshape(C, 9 * C)
    ws128 = np.concatenate([ws, ws], axis=0).astype(np.float16)

    # b2' absorbs the {0,1}-encoding correction: -0.5*S2*sum(sign W)
    b2p = b2 - 0.5 * S2 * wsign.sum(axis=(1, 2, 3))
    bi = np.concatenate([b2p, b2p]).astype(np.float32)[:, None]  # [128, 1]
    packed = np.concatenate(
        [
            np.ascontiguousarray(ws128).view(np.uint8),
            np.ascontiguousarray(bi).view(np.uint8),
        ],
        axis=1,
    )  # [128, 1156]
    return packed


_CACHE = {}


def kernel(x, weight, bias, gamma, beta, run_mean, run_var, _trace=False, _trace_kwargs=None):
    x = np.asarray(x)
    consts = prep_consts(weight, bias, gamma, beta, run_mean, run_var)
    # [core][pair][ih*64+c][h*w] in fp16
    x16 = np.ascontiguousarray(
        x.reshape(NCORES, BSH // 2, 128, HWF).astype(np.float16)
    )

    if "nc" not in _CACHE:
        _CACHE["nc"] = build_nc()
    nc = _CACHE["nc"]

    in_maps = [dict(x=x16[i], consts=consts) for i in range(NCORES)]
    res = bass_utils.run_bass_kernel_spmd(
        nc,
        in_maps,
        core_ids=list(range(NCORES)),
        trace=_trace,
        **(_trace_kwargs or {}),
    )
    outs = []
    for i in range(NCORES):
        o = np.asarray(res.results[i]["out"])  # [4, 128, 6272] fp16
        # partition=(hf,c), free=(m,j,r,w); row = m*16 + hf*8 + j*4 + r
        o = (
            o.reshape(BSH, 2, C, 7, 2, 4, W)
            .transpose(0, 2, 3, 1, 4, 5, 6)
            .reshape(BSH, C, H, W)
        )
        outs.append(o)
    out = np.concatenate(outs, axis=0).astype(np.float32)
    if _trace:
        kernel.last_results = res
    return out
